# revision 4
# baseline (speedup 1.0000x reference)
"""GQA attention (32 q-heads / 8 kv-heads, HD=128, S=2048, sliding window 1024)
on 8 Trainium2 NeuronCores.

Sharding: tensor-parallel over heads — core c owns kv-head c and q-heads
4c..4c+3. Each core computes its QKV projections (f32r matmuls), RoPE,
windowed attention in a transposed layout (scores kept as [k, q] so softmax
denominators come from an M=1 matmul and PV needs no transposes), and its
partial o_proj. Host sums the 8 o_proj partials (the "all-reduce") and
stacks k/v slices.

All matmuls run in float32r (11-bit mantissa, 1 PE cycle/row at N>=256).
Weights and x are pre-rounded to f32r on the host for free.
"""
import sys

sys.path.insert(0, "/opt/trn_rl_repo")

import numpy as np
import concourse.bacc as bacc
import concourse.tile as tile
import concourse.mybir as mybir
from concourse.bass_utils import run_bass_kernel_spmd

dt = mybir.dt
AF = mybir.ActivationFunctionType

B, S, HID = 1, 2048, 4096
NH, NKV, HD = 32, 8, 128
G = NH // NKV          # q heads per core
WINDOW = 1024
ROPE_BASE = 10000.0
N_CORES = 8
SCALE = 1.0 / np.sqrt(HD)
SQRT_HD = float(np.sqrt(HD))

SPAN = 256             # seq span per QKV/attention step
NSPAN = S // SPAN      # 8
KCH = HID // 128       # 32 contraction chunks
QSEG = G * HD          # 512 head dims per core

LAST_RESULTS = None
LAST_EXEC_NS = None
LAST_IN_MAPS = None


def round_f32r(x: np.ndarray) -> np.ndarray:
    """Round fp32 to f32r (11-bit mantissa, round-to-nearest-even)."""
    u = np.ascontiguousarray(x, dtype=np.float32).view(np.uint32).astype(np.uint64)
    r = ((u + (1 << 11) - 1 + ((u >> 12) & 1)) >> 12) << 12
    return r.astype(np.uint32).view(np.float32)


def _rope_cos_sin():
    inv_freq = 1.0 / (ROPE_BASE ** (np.arange(0, HD, 2, dtype=np.float32) / HD))
    t = np.arange(S, dtype=np.float32)
    freqs = t[:, None] * inv_freq[None, :]
    emb = np.concatenate([freqs, freqs], axis=-1)        # [S, HD]
    return np.cos(emb).astype(np.float32), np.sin(emb).astype(np.float32)


def _build_masks():
    """[128, 256] multiplicative masks for boundary kj tiles, [k, q] layout.

    Within a span of 256 queries (two 128-tiles qi_left/qi_right) and one
    128-tile of keys kj:
      T1 (kj == qi_right):      [zero | diag]
      T2 (kj == qi_left):       [diag | ones]
      T3 (kj == qi_left - 7):   [ones | far]
      T4 (kj == qi_left - 8):   [far  | zero]
    diag: allow q_in_tile >= k_in_tile; far: allow q_in_tile <= k_in_tile.
    """
    jj = np.arange(128)[:, None]
    ii = np.arange(128)[None, :]
    diag = (ii >= jj).astype(np.float32)
    far = (ii <= jj).astype(np.float32)
    ones = np.ones((128, 128), np.float32)
    zero = np.zeros((128, 128), np.float32)
    t1 = np.concatenate([zero, diag], 1)
    t2 = np.concatenate([diag, ones], 1)
    t3 = np.concatenate([ones, far], 1)
    t4 = np.concatenate([far, zero], 1)
    return t1, t2, t3, t4


def _build_program():
    nc = bacc.Bacc("TRN2", target_bir_lowering=False, debug=False,
                   num_devices=N_CORES)

    f32, f32r = dt.float32, dt.float32r

    xT_d = nc.dram_tensor("xT", [HID, S], f32r, kind="ExternalInput").ap()
    wqT_d = nc.dram_tensor("wqT", [HID, QSEG], f32r, kind="ExternalInput").ap()
    wkT_d = nc.dram_tensor("wkT", [HID, HD], f32r, kind="ExternalInput").ap()
    wvT_d = nc.dram_tensor("wvT", [HID, HD], f32r, kind="ExternalInput").ap()
    woT_d = nc.dram_tensor("woT", [QSEG, HID], f32r, kind="ExternalInput").ap()
    cosq_d = nc.dram_tensor("cosq", [HD, S], f32, kind="ExternalInput").ap()
    sinq_d = nc.dram_tensor("sinq", [HD, S], f32, kind="ExternalInput").ap()
    mask_d = [nc.dram_tensor(f"maskT{i}", [128, 256], f32,
                             kind="ExternalInput").ap() for i in range(1, 5)]
    ident_d = nc.dram_tensor("ident", [128, 128], f32r, kind="ExternalInput").ap()
    onec_d = nc.dram_tensor("onec", [128, 1], f32r, kind="ExternalInput").ap()
    oner_d = nc.dram_tensor("oner", [1, 128], f32r, kind="ExternalInput").ap()

    partialT_d = nc.dram_tensor("partialT", [HID, S], f32,
                                kind="ExternalOutput").ap()
    kT_out_d = nc.dram_tensor("kT_out", [HD, S], f32, kind="ExternalOutput").ap()
    vT_out_d = nc.dram_tensor("vT_out", [HD, S], f32, kind="ExternalOutput").ap()

    from contextlib import ExitStack
    with tile.TileContext(nc) as tc, ExitStack() as ctx, \
         nc.allow_low_precision(reason="f32r matmul intermediates"):
        const = ctx.enter_context(tc.tile_pool(name="const", bufs=1))
        persist = ctx.enter_context(tc.tile_pool(name="persist", bufs=1))

        ident = const.tile([128, 128], f32r, tag="ident")
        onec = const.tile([128, 1], f32r, tag="onec")
        oner = const.tile([1, 128], f32r, tag="oner")
        masks = [const.tile([128, 256], f32, tag=f"mask{i}", name=f"mask{i}")
                 for i in range(4)]
        nc.sync.dma_start(ident[:], ident_d[:])
        nc.sync.dma_start(onec[:], onec_d[:])
        nc.sync.dma_start(oner[:], oner_d[:])
        for i in range(4):
            nc.sync.dma_start(masks[i][:], mask_d[i][:])

        # persistent activations
        qrope = persist.tile([128, G * S], f32r, tag="qrope")   # [HD, h*S+seq]
        krope_r = persist.tile([HD, S], f32r, tag="krope_r")
        vtr = persist.tile([128, S], f32r, tag="vtr")           # v transposed: [seq%128, 128*t+hd]
        outT = persist.tile([128, G * S], f32r, tag="outT")     # [HD, h*S+seq]

        # ---------------- Phase 1: QKV projections + RoPE ----------------
        with tc.tile_pool(name="wts", bufs=1) as wpool:
            wq_sb = wpool.tile([128, KCH * QSEG], f32r, tag="wq")
            wk_sb = wpool.tile([128, KCH * HD], f32r, tag="wk")
            wv_sb = wpool.tile([128, KCH * HD], f32r, tag="wv")
            for k in range(KCH):
                nc.sync.dma_start(wq_sb[:, k * QSEG:(k + 1) * QSEG],
                                  wqT_d[k * 128:(k + 1) * 128, :])
                nc.sync.dma_start(wk_sb[:, k * HD:(k + 1) * HD],
                                  wkT_d[k * 128:(k + 1) * 128, :])
                nc.sync.dma_start(wv_sb[:, k * HD:(k + 1) * HD],
                                  wvT_d[k * 128:(k + 1) * 128, :])

            with tc.tile_pool(name="xq", bufs=4) as xqp, \
                 tc.tile_pool(name="qkvp", bufs=7, space="PSUM") as pp, \
                 tc.tile_pool(name="tpp", bufs=1, space="PSUM") as tpp, \
                 tc.tile_pool(name="rsc", bufs=2) as rsc:
                for s in range(NSPAN):
                    sl = slice(s * SPAN, (s + 1) * SPAN)
                    pq = [pp.tile([128, SPAN], f32, tag="ps", name=f"pq{_h}")
                          for _h in range(G)]
                    pk = pp.tile([128, SPAN], f32, tag="ps")
                    pv = pp.tile([128, SPAN], f32, tag="ps")
                    for k in range(KCH):
                        xq = xqp.tile([128, SPAN], f32r, tag="xq")
                        nc.sync.dma_start(xq[:], xT_d[k * 128:(k + 1) * 128, sl])
                        st, sp = (k == 0), (k == KCH - 1)
                        for h in range(G):
                            nc.tensor.matmul(
                                pq[h][:],
                                wq_sb[:, k * QSEG + h * HD: k * QSEG + (h + 1) * HD],
                                xq[:], start=st, stop=sp)
                        nc.tensor.matmul(pk[:], wk_sb[:, k * HD:(k + 1) * HD],
                                         xq[:], start=st, stop=sp)
                        nc.tensor.matmul(pv[:], wv_sb[:, k * HD:(k + 1) * HD],
                                         xq[:], start=st, stop=sp)

                    cq = rsc.tile([128, SPAN], f32, tag="cq")
                    sq = rsc.tile([128, SPAN], f32, tag="sq")
                    nc.sync.dma_start(cq[:], cosq_d[:, sl])
                    nc.sync.dma_start(sq[:], sinq_d[:, sl])
                    # k-rope cos/sin = q's unscaled
                    cosk = rsc.tile([128, SPAN], f32, tag="cosk")
                    sink = rsc.tile([128, SPAN], f32, tag="sink")
                    nc.vector.tensor_scalar_mul(cosk[:], cq[:], SQRT_HD)
                    nc.vector.tensor_scalar_mul(sink[:], sq[:], SQRT_HD)

                    def rope(psrc, cos_ap, sin_ap, out_ap):
                        t1 = rsc.tile([128, SPAN], f32, tag="t1")
                        t2 = rsc.tile([128, SPAN], f32, tag="t2")
                        nc.vector.tensor_mul(t1[:], psrc[:], cos_ap)
                        nc.vector.tensor_mul(t2[0:64, :], psrc[64:128, :],
                                             sin_ap[0:64, :])
                        nc.vector.tensor_mul(t2[64:128, :], psrc[0:64, :],
                                             sin_ap[64:128, :])
                        nc.vector.tensor_add(out_ap, t1[:], t2[:])

                    for h in range(G):
                        rope(pq[h], cq[:], sq[:],
                             qrope[:, h * S + s * SPAN: h * S + (s + 1) * SPAN])
                    rope(pk, cosk[:], sink[:], krope_r[:, sl])
                    nc.sync.dma_start(kT_out_d[:, sl],
                                      krope_r[:, sl].bitcast(f32))

                    vf = rsc.tile([128, SPAN], f32, tag="vf")
                    nc.scalar.copy(vf[:], pv[:])
                    nc.sync.dma_start(vT_out_d[:, sl], vf[:])
                    vr = rsc.tile([128, SPAN], f32r, tag="vr")
                    nc.scalar.copy(vr[:], pv[:])
                    for half in range(2):
                        tp = tpp.tile([128, 128], f32r, tag="tp")
                        nc.tensor.transpose(tp[:], vr[:, half * 128:(half + 1) * 128],
                                            ident[:])
                        t0 = 2 * s + half
                        nc.scalar.copy(vtr[:, t0 * 128:(t0 + 1) * 128], tp[:])

        # ---------------- Phase 2: windowed attention ----------------
        with tc.tile_pool(name="scp", bufs=3, space="PSUM") as scp_p, \
             tc.tile_pool(name="pvp", bufs=2, space="PSUM") as pvp_p, \
             tc.tile_pool(name="dnp", bufs=2, space="PSUM") as dnp_p, \
             tc.tile_pool(name="bcp", bufs=1, space="PSUM") as bcp_p, \
             tc.tile_pool(name="pts", bufs=14) as pts_p, \
             tc.tile_pool(name="att_sc", bufs=3) as asc:
            for h in range(G):
                qbase = h * S
                for s in range(NSPAN):
                    qsl = slice(qbase + s * SPAN, qbase + (s + 1) * SPAN)
                    kjs = list(range(max(0, 2 * s - 8), 2 * s + 2))
                    pts = []
                    for kj in kjs:
                        sc = scp_p.tile([128, SPAN], f32, tag="sc")
                        nc.tensor.matmul(sc[:],
                                         krope_r[:, kj * 128:(kj + 1) * 128],
                                         qrope[:, qsl], start=True, stop=True)
                        pt = pts_p.tile([128, SPAN], f32r, tag="pt")
                        nc.scalar.activation(pt[:], sc[:], AF.Exp)
                        if kj == 2 * s + 1:
                            nc.vector.tensor_mul(pt[:], pt[:], masks[0][:])
                        elif kj == 2 * s:
                            nc.vector.tensor_mul(pt[:], pt[:], masks[1][:])
                        elif kj == 2 * s - 7:
                            nc.vector.tensor_mul(pt[:], pt[:], masks[2][:])
                        elif kj == 2 * s - 8:
                            nc.vector.tensor_mul(pt[:], pt[:], masks[3][:])
                        pts.append(pt)

                    dn = dnp_p.tile([1, SPAN], f32, tag="dn")
                    pv = pvp_p.tile([128, SPAN], f32, tag="pv")
                    for i, (kj, pt) in enumerate(zip(kjs, pts)):
                        st, sp = (i == 0), (i == len(kjs) - 1)
                        nc.tensor.matmul(dn[:], onec[:], pt[:], start=st, stop=sp)
                        nc.tensor.matmul(pv[:], vtr[:, kj * 128:(kj + 1) * 128],
                                         pt[:], start=st, stop=sp)

                    rc = asc.tile([1, SPAN], f32r, tag="rc")
                    nc.vector.reciprocal(rc[:], dn[:])
                    bc = bcp_p.tile([128, SPAN], f32, tag="bc")
                    nc.tensor.matmul(bc[:], oner[:], rc[:], start=True, stop=True)
                    bcs = asc.tile([128, SPAN], f32, tag="bcs")
                    nc.scalar.copy(bcs[:], bc[:])
                    nc.vector.tensor_mul(outT[:, qsl], pv[:], bcs[:])

        # ---------------- Phase 3: o_proj partial ----------------
        with tc.tile_pool(name="wo", bufs=8) as wop, \
             tc.tile_pool(name="opp", bufs=4, space="PSUM") as opp, \
             tc.tile_pool(name="oev", bufs=4) as oev:
            NT = S // 512  # 4 seq tiles of 512
            for m in range(HID // 128):
                wts = []
                for kk in range(G):
                    w = wop.tile([128, 128], f32r, tag="w", name=f"wo{m}_{kk}")
                    nc.sync.dma_start(
                        w[:], woT_d[kk * 128:(kk + 1) * 128,
                                    m * 128:(m + 1) * 128])
                    wts.append(w)
                for n in range(NT):
                    ps = opp.tile([128, 512], f32, tag="op")
                    for kk in range(G):
                        nc.tensor.matmul(ps[:], wts[kk][:],
                                         outT[:, kk * S + n * 512:
                                              kk * S + (n + 1) * 512],
                                         start=(kk == 0), stop=(kk == G - 1))
                    ev = oev.tile([128, 512], f32, tag="ev")
                    nc.scalar.copy(ev[:], ps[:])
                    nc.sync.dma_start(
                        partialT_d[m * 128:(m + 1) * 128,
                                   n * 512:(n + 1) * 512], ev[:])

    nc.compile()
    return nc


_nc_cache = None


def _get_program():
    global _nc_cache
    if _nc_cache is None:
        _nc_cache = _build_program()
    return _nc_cache


def kernel(hidden_states, Wq, Wk, Wv, Wo):
    global LAST_RESULTS, LAST_EXEC_NS, LAST_IN_MAPS
    hidden_states = np.asarray(hidden_states, dtype=np.float32)
    Wq = np.asarray(Wq, dtype=np.float32)
    Wk = np.asarray(Wk, dtype=np.float32)
    Wv = np.asarray(Wv, dtype=np.float32)
    Wo = np.asarray(Wo, dtype=np.float32)

    nc = _get_program()

    x = hidden_states[0]                                  # [S, HID]
    xT = round_f32r(x.T)
    cos, sin = _rope_cos_sin()                            # [S, HD]
    sign = np.concatenate([-np.ones(HD // 2), np.ones(HD // 2)]).astype(np.float32)
    cosqT = np.ascontiguousarray(cos.T) * np.float32(SCALE)
    sinqT = np.ascontiguousarray((sin * sign[None, :]).T) * np.float32(SCALE)
    m1, m2, m3, m4 = _build_masks()
    ident = round_f32r(np.eye(128, dtype=np.float32))
    onec = np.ones((128, 1), np.float32)
    oner = np.ones((1, 128), np.float32)

    in_maps = []
    for c in range(N_CORES):
        qsl = slice(c * QSEG, (c + 1) * QSEG)
        ksl = slice(c * HD, (c + 1) * HD)
        in_maps.append({
            "xT": xT,
            "wqT": round_f32r(Wq[qsl, :].T),
            "wkT": round_f32r(Wk[ksl, :].T),
            "wvT": round_f32r(Wv[ksl, :].T),
            "woT": round_f32r(Wo[:, qsl].T),
            "cosq": cosqT, "sinq": sinqT,
            "maskT1": m1, "maskT2": m2, "maskT3": m3, "maskT4": m4,
            "ident": ident, "onec": onec, "oner": oner,
        })

    LAST_IN_MAPS = in_maps
    res = run_bass_kernel_spmd(nc, in_maps, list(range(N_CORES)))
    LAST_RESULTS = res
    LAST_EXEC_NS = res.exec_time_ns

    acc = np.zeros((HID, S), np.float64)
    for c in range(N_CORES):
        acc += res.results[c]["partialT"]
    attn = acc.T.astype(np.float32)[None]                 # [1, S, HID]

    k_out = np.stack([res.results[c]["kT_out"].T for c in range(N_CORES)])[None]
    v_out = np.stack([res.results[c]["vT_out"].T for c in range(N_CORES)])[None]
    return attn, k_out.astype(np.float32), v_out.astype(np.float32)
